# revision 3
# baseline (speedup 1.0000x reference)
"""DeepNCM forward (vq_codebook) on 8 TRN2 NeuronCores — v2.

Data-parallel over N=32768 rows (4096/core).  Host pre-shards and lays out
(dtype casts + transposes only; all FLOPs on device):
  emb8  [128, 32, 1024] fp8  n-major rows   (segsum lhsT, esq source)
  embT8 [128, 8*4096]   fp8  d-major        (distance lhsT)
  ptT   [128, 8*1024]   f16  transposed prototypes (replicated)
  counter [1024] f16 (replicated), y [4096] i32

v2 vs baseline:
  - embf (bf16 embeddings, 8MB/core) dropped: esq from fp8 emb8 on ScalarE
    (adds ~9e-3 rel err; total 1.2e-2 < 2e-2 budget).
  - esq staged across windows (tiles 0-15 in sweep A, 16-23 in sweep B,
    24-31 before the AR-dependent ScalarE work) so no engine queue stalls
    head-of-line on an AllReduce.
  - epilogue: psq folded into PSUM by a K=1 ones@psqn matmul per half;
    esq applied via the scalar slot of a single evacuation op per tile
    (DVE subtract for even tiles, ScalarE activation-bias for odd).
  - phase 2 software-pipelined around the 4-deep PSUM pool: j0-j2 of
    tiles i+1..i+4 interleave with j3+corr of tile i, hiding AR-C.
  - update row math in f16 with a = 2(1-m) + ctr*b and a divide op.
  - DMAs only on the otherwise-idle sync/gpsimd queues.
"""
import sys

sys.path.insert(0, "/opt/trn_rl_repo")

import numpy as np
import ml_dtypes
import concourse.bass as bass
import concourse.bacc as bacc
import concourse.tile as tile
import concourse.mybir as mybir
from concourse import bass_utils

F32 = mybir.dt.float32
F16 = mybir.dt.float16
BF16 = mybir.dt.bfloat16
F8 = mybir.dt.float8e4
I32 = mybir.dt.int32
I16 = mybir.dt.int16
AOT = mybir.AluOpType
ACTF = mybir.ActivationFunctionType
DR = mybir.MatmulPerfMode.DoubleRow

MOCK_CC = False  # replace collectives with local copies (sim-timing probe)

N_CORES = 8
N_FULL = 32768
C = 1024
D = 1024
N_SHARD = N_FULL // N_CORES  # 4096
NT = N_SHARD // 128          # 32 row tiles per core
NP = NT // 2                 # 16 DoubleRow pair groups
KB = D // 128                # 8 d-blocks
SWEEPS = [(0, 3), (3, 7), (7, 8)]  # k-ranges of the three AR chunks


def build(repeat=1):
    nc = bacc.Bacc("TRN2", target_bir_lowering=False, debug=False,
                   num_devices=N_CORES)
    emb8d = nc.dram_tensor("emb8", [N_SHARD, D], F8, kind="ExternalInput").ap()
    embT8d = nc.dram_tensor("embT8", [128, KB * N_SHARD], F8,
                            kind="ExternalInput").ap()
    y = nc.dram_tensor("y", [N_SHARD], I32, kind="ExternalInput").ap()
    ptTd = nc.dram_tensor("ptT", [128, KB * C], F16,
                          kind="ExternalInput").ap()
    counterd = nc.dram_tensor("counter", [C], F16, kind="ExternalInput").ap()
    out = nc.dram_tensor("out", [N_SHARD, C], F16, kind="ExternalOutput").ap()

    with tile.TileContext(nc) as tc:
        with tc.tile_pool(name="resid", bufs=1) as resid, \
             tc.tile_pool(name="dram", bufs=1, space="DRAM") as dramp, \
             tc.tile_pool(name="outp", bufs=3) as outp:

            # ---- constants / small residents ----
            iota = resid.tile([128, C], I16, tag="iota")
            nc.gpsimd.iota(iota, pattern=[[1, C]], base=0, channel_multiplier=0)
            ones8 = resid.tile([128, 32], F8, tag="ones8")
            nc.vector.memset(ones8, 1.0)
            ones8dr = ones8.rearrange("p (a b) -> p a b", b=16)[:, :, 0:1]
            ones_row = resid.tile([1, 128], BF16, tag="ones_row")
            nc.vector.memset(ones_row, 1.0)
            ones_f16 = resid.tile([1, 128], F16, tag="ones_f16")
            nc.vector.memset(ones_f16, 1.0)
            ones_mat = resid.tile([128, 128], F16, tag="ones_mat")
            nc.vector.memset(ones_mat, 1.0)
            y_i32 = resid.tile([128, NT], I32, tag="y_i32")
            nc.sync.dma_start(y_i32, y.rearrange("(n p) -> p n", p=128))
            y_f32 = resid.tile([128, NT], F32, tag="y_f32")
            nc.vector.tensor_copy(y_f32, y_i32)
            esq = resid.tile([128, NT], F32, tag="esq")
            esqn = resid.tile([128, NT], F32, tag="esqn")
            ctr16 = resid.tile([1, C], F16, tag="ctr16")
            nc.sync.dma_start(ctr16, counterd.rearrange("(a b) -> a b", a=1))

            # big residents
            emb8 = resid.tile([128, NT, D], F8, tag="emb8")       # n-major
            embT8 = resid.tile([128, KB, N_SHARD], F8, tag="embT8")  # d-major
            ohc = resid.tile([128, NT, C], F8, tag="ohc")         # one-hots
            ptT = resid.tile([128, KB, C], F16, tag="ptT")        # protoT
            st = resid.tile([128, KB, C], F16, tag="st")          # AR'd sumsT
            np8 = resid.tile([128, KB, C], F8, tag="np8")         # 2*newprotoT
            b_bc = resid.tile([128, C], F16, tag="b_bc")
            a_bc = resid.tile([128, C], F16, tag="a_bc")
            psqn = resid.tile([1, C], BF16, tag="psqn")           # -psq row

            for rep in range(repeat):
                ar_in = []
                ar_out = []
                for k in range(KB):
                    ar_in.append(dramp.tile([128, C], F8,
                                            name=f"ar_in{k}_{rep}"))
                    ar_out.append(dramp.tile([128, C], F8,
                                             name=f"ar_out{k}_{rep}",
                                             addr_space="Shared"))
                arc_in = dramp.tile([1, C], F16, name=f"arc_in_{rep}")
                arc_out = dramp.tile([1, C], F16, name=f"arc_out_{rep}",
                                     addr_space="Shared")

                def issue_ar(k):
                    if MOCK_CC:
                        nc.gpsimd.dma_start(ar_out[k], ar_in[k])
                    else:
                        nc.gpsimd.collective_compute(
                            "AllReduce", AOT.add,
                            ins=[ar_in[k].opt()], outs=[ar_out[k].opt()],
                            replica_groups=[list(range(N_CORES))],
                        )

                # resident loads: emb8 first (phase 1), embT8/ptT behind
                for c4 in range(8):
                    e = nc.sync if c4 % 2 == 0 else nc.gpsimd
                    e.dma_start(emb8[:, 4 * c4:4 * (c4 + 1), :],
                                emb8d[:, :].rearrange("(n p) d -> p n d", p=128)
                                [:, 4 * c4:4 * (c4 + 1), :])
                nc.gpsimd.dma_start(ptT[:, :, :],
                                    ptTd.rearrange("p (k c) -> p k c", k=KB))

                # ---- phase 1 sweep A: one-hots, esq 0-15, counts, k0-2 ----
                with tc.tile_pool(name=f"scr{rep}", bufs=2) as scrp, \
                     tc.tile_pool(name=f"flush{rep}", bufs=4) as flp, \
                     tc.tile_pool(name=f"sqp{rep}", bufs=2) as sqp:

                    pswA = tc.alloc_tile_pool(name=f"pswA{rep}", bufs=1,
                                              space="PSUM")
                    ps_a = [[pswA.tile([128, 512], F32, tag=f"psA{k}{h}",
                                       name=f"psA_{k}_{h}_{rep}")
                             for h in range(2)] for k in range(3)]
                    ps_c = [pswA.tile([1, 512], F32, tag=f"pc{h}",
                                      name=f"psc{h}_{rep}") for h in range(2)]

                    for p in range(NP):
                        for t in (2 * p, 2 * p + 1):
                            nc.vector.tensor_scalar(ohc[:, t, :], iota,
                                                    y_f32[:, t:t + 1], None,
                                                    op0=AOT.is_equal)
                        if p < 8:  # esq tiles 0..15 during sweep A
                            for t in (2 * p, 2 * p + 1):
                                scr8 = scrp.tile([128, D], F8, tag="scr8")
                                nc.scalar.activation(scr8, emb8[:, t, :],
                                                     ACTF.Square,
                                                     accum_out=esq[:, t:t + 1])
                        for h in range(2):
                            nc.tensor.matmul(
                                ps_c[h], ones8dr,
                                ohc[:, 2 * p:2 * p + 2, h * 512:(h + 1) * 512],
                                start=(p == 0), stop=(p == NP - 1),
                                perf_mode=DR)
                        for k in range(0, 3):
                            for h in range(2):
                                nc.tensor.matmul(
                                    ps_a[k][h],
                                    emb8[:, 2 * p:2 * p + 2,
                                         k * 128:(k + 1) * 128],
                                    ohc[:, 2 * p:2 * p + 2,
                                        h * 512:(h + 1) * 512],
                                    start=(p == 0), stop=(p == NP - 1),
                                    perf_mode=DR)

                    # counts AR first (tiny, unblocks the row math), then
                    # per-k fp8 AllReduces
                    flc = flp.tile([1, C], F16, tag="flc")
                    nc.scalar.copy(flc[:, 0:512], ps_c[0])
                    nc.vector.tensor_copy(flc[:, 512:1024], ps_c[1])
                    nc.sync.dma_start(arc_in, flc)
                    if MOCK_CC:
                        nc.gpsimd.dma_start(arc_out, arc_in)
                    else:
                        nc.gpsimd.collective_compute(
                            "AllReduce", AOT.add,
                            ins=[arc_in.opt()], outs=[arc_out.opt()],
                            replica_groups=[list(range(N_CORES))],
                        )
                    for k in range(0, 3):
                        fl = flp.tile([128, C], F8, tag="fl8")
                        nc.scalar.copy(fl[:, 0:512], ps_a[k][0])
                        nc.vector.tensor_copy(fl[:, 512:1024], ps_a[k][1])
                        nc.sync.dma_start(ar_in[k][0:128, :], fl)
                        issue_ar(k)
                    pswA.release()
                    cnt16 = resid.tile([1, C], F16, tag="cnt16",
                                       name=f"cnt16_{rep}")
                    nc.sync.dma_start(cnt16, arc_out)

                    # ---- sweep B (k3-6), k-outer; esq 16-23 (ScalarE) ----
                    pswB = tc.alloc_tile_pool(name=f"pswB{rep}", bufs=1,
                                              space="PSUM")
                    ps_b = [[pswB.tile([128, 512], F32, tag=f"psB{k}{h}",
                                       name=f"psB_{k}_{h}_{rep}")
                             for h in range(2)] for k in range(4)]
                    for k in range(3, 7):
                        for p in range(NP):
                            for h in range(2):
                                nc.tensor.matmul(
                                    ps_b[k - 3][h],
                                    emb8[:, 2 * p:2 * p + 2,
                                         k * 128:(k + 1) * 128],
                                    ohc[:, 2 * p:2 * p + 2,
                                        h * 512:(h + 1) * 512],
                                    start=(p == 0), stop=(p == NP - 1),
                                    perf_mode=DR)
                        t0 = 16 + 2 * (k - 3)
                        for t in (t0, t0 + 1):
                            scr8 = scrp.tile([128, D], F8, tag="scr8")
                            nc.scalar.activation(scr8, emb8[:, t, :],
                                                 ACTF.Square,
                                                 accum_out=esq[:, t:t + 1])
                        fl = flp.tile([128, C], F8, tag="fl8")
                        nc.scalar.copy(fl[:, 0:512], ps_b[k - 3][0])
                        nc.vector.tensor_copy(fl[:, 512:1024], ps_b[k - 3][1])
                        nc.sync.dma_start(ar_in[k][0:128, :], fl)
                        issue_ar(k)
                    pswB.release()

                    # ---- rows math on DVE (f32 ops, in-place reuse) ----
                    with tc.tile_pool(name=f"upd{rep}", bufs=1) as updp:
                        cnt32 = updp.tile([1, C], F32, tag="cnt32")
                        nc.vector.tensor_copy(cnt32, cnt16)
                        ctr32 = updp.tile([1, C], F32, tag="ctr32")
                        nc.vector.tensor_copy(ctr32, ctr16)
                        m32 = updp.tile([1, C], F32, tag="m32")
                        nc.vector.tensor_scalar(m32, cnt32, 0.0, None,
                                                op0=AOT.is_gt)
                        # cnt32 <- max(ctr+cnt, 1)
                        nc.vector.tensor_tensor(cnt32, ctr32, cnt32,
                                                op=AOT.add)
                        nc.vector.tensor_scalar(cnt32, cnt32, 1.0, None,
                                                op0=AOT.max)
                        inv = updp.tile([1, C], F32, tag="inv")
                        nc.vector.reciprocal(inv, cnt32)
                        # inv <- m/tot
                        nc.vector.tensor_tensor(inv, inv, m32, op=AOT.mult)
                        brow = updp.tile([1, C], F16, tag="brow")
                        nc.vector.tensor_scalar(brow, inv, 2.0, None,
                                                op0=AOT.mult)
                        # m32 <- 2(1-m);  ctr32 <- 2*ctr*m/tot
                        nc.vector.tensor_scalar(m32, m32, -2.0, 2.0,
                                                op0=AOT.mult, op1=AOT.add)
                        nc.vector.tensor_tensor(ctr32, ctr32, inv,
                                                op=AOT.mult)
                        nc.vector.tensor_scalar(ctr32, ctr32, 2.0, None,
                                                op0=AOT.mult)
                        arow = updp.tile([1, C], F16, tag="arow")
                        nc.vector.tensor_tensor(arow, m32, ctr32, op=AOT.add)

                        # broadcast b then a (PE K=1, DVE evac; b first so
                        # the st multiplies can start earliest)
                        psab = tc.alloc_tile_pool(name=f"psab{rep}", bufs=1,
                                                  space="PSUM")
                        for j, (row, dst) in enumerate(((brow, b_bc),
                                                        (arow, a_bc))):
                            for h in range(2):
                                pab = psab.tile([128, 512], F32,
                                                tag=f"pab{j}{h}",
                                                name=f"pab{j}{h}_{rep}")
                                nc.tensor.matmul(pab, ones_f16,
                                                 row[:, h * 512:(h + 1) * 512],
                                                 start=True, stop=True)
                                nc.vector.tensor_copy(
                                    dst[:, h * 512:(h + 1) * 512], pab)
                        psab.release()

                    # ---- sweep C (k7): PE; flush on ScalarE ----
                    pswC = tc.alloc_tile_pool(name=f"pswC{rep}", bufs=1,
                                              space="PSUM")
                    ps_cc = [pswC.tile([128, 512], F32, tag=f"psC{h}",
                                       name=f"psC_{h}_{rep}") for h in range(2)]
                    for p in range(NP):
                        for h in range(2):
                            nc.tensor.matmul(
                                ps_cc[h],
                                emb8[:, 2 * p:2 * p + 2, 7 * 128:8 * 128],
                                ohc[:, 2 * p:2 * p + 2, h * 512:(h + 1) * 512],
                                start=(p == 0), stop=(p == NP - 1),
                                perf_mode=DR)
                    fl = flp.tile([128, C], F8, tag="fl8")
                    nc.scalar.copy(fl[:, 0:512], ps_cc[0])
                    nc.scalar.copy(fl[:, 512:1024], ps_cc[1])
                    nc.sync.dma_start(ar_in[7][0:128, :], fl)
                    issue_ar(7)
                    pswC.release()

                    # esq tiles 24..27 now; 28..31 interleave with evacs
                    for t in range(24, 28):
                        scr8 = scrp.tile([128, D], F8, tag="scr8")
                        nc.scalar.activation(scr8, emb8[:, t, :], ACTF.Square,
                                             accum_out=esq[:, t:t + 1])

                    # embT8 loads: needed only by phase 2; keep their DMA
                    # traffic behind the AllReduce path
                    for c4 in range(4):
                        e = nc.sync if c4 % 2 == 0 else nc.gpsimd
                        e.dma_start(embT8[:, 2 * c4:2 * (c4 + 1), :],
                                    embT8d[:, 2 * c4 * N_SHARD:
                                           2 * (c4 + 1) * N_SHARD]
                                    .rearrange("p (k n) -> p k n", k=2))

                    # ---- updates per k: st <- b(.)st, ptT_k <- a(.)ptT_k,
                    #      np8 = st + a.ptT (DVE); squares (ScalarE);
                    #      psq accumulation (PE) ----
                    psp = tc.alloc_tile_pool(name=f"psp{rep}", bufs=3,
                                             space="PSUM")
                    psup = tc.alloc_tile_pool(name=f"psup{rep}", bufs=1,
                                              space="PSUM")
                    ps_psq = [psup.tile([128, 512], F32, tag=f"psq{h}",
                                        name=f"pspsq{h}_{rep}")
                              for h in range(2)]
                    sq7 = None
                    for s, (k0, k1) in enumerate(SWEEPS):
                        for k in range(k0, k1):
                            st8 = sqp.tile([128, C], F8, tag="st8")
                            nc.sync.dma_start(
                                st8,
                                ar_out[k][0:128, :].rearrange(
                                    "(a p) c -> p (a c)", p=128))
                            if k % 2 == 0:
                                nc.scalar.activation(st[:, k, :], st8,
                                                     ACTF.Identity)
                            else:
                                nc.vector.tensor_copy(st[:, k, :], st8)
                            nc.vector.tensor_tensor(st[:, k, :], st[:, k, :],
                                                    b_bc, op=AOT.mult)
                            nc.vector.tensor_tensor(ptT[:, k, :], ptT[:, k, :],
                                                    a_bc, op=AOT.mult)
                            nc.vector.tensor_tensor(np8[:, k, :], st[:, k, :],
                                                    ptT[:, k, :], op=AOT.add)
                            sq = sqp.tile([128, C], F16, tag="sq")
                            nc.scalar.activation(sq, np8[:, k, :], ACTF.Square)
                            if k < 7:  # k7's psq matmuls deferred past prefill
                                for h in range(2):
                                    nc.tensor.matmul(
                                        ps_psq[h], ones_mat,
                                        sq[:, h * 512:(h + 1) * 512],
                                        start=(k == 0), stop=False)
                            else:
                                sq7 = sq
                    # negated esq for the ScalarE (bias) evacs, tiles 0-23
                    nc.vector.tensor_scalar(esqn[:, 0:24], esq[:, 0:24], -1.0,
                                            None, op0=AOT.mult)

                    # ---- phase 2: software-pipelined distances ----
                    psp2 = [None]
                    pds = {}

                    def open_tile(i):
                        pool = psp2[0] if i % 4 == 3 else psp
                        pd = pool.tile([128, C], F32, tag="pd",
                                       name=f"pd{i}_{rep}")
                        pds[i] = pd
                        for h in range(2):
                            for j in range(3):
                                nc.tensor.matmul(
                                    pd[:, h * 512:(h + 1) * 512],
                                    embT8[:, 2 * j:2 * j + 2,
                                          i * 128:(i + 1) * 128],
                                    np8[:, 2 * j:2 * j + 2,
                                        h * 512:(h + 1) * 512],
                                    start=(j == 0), stop=False,
                                    perf_mode=DR)

                    for i in range(3):
                        open_tile(i)
                    # deferred k7 psq accumulation + psqn extraction
                    for h in range(2):
                        nc.tensor.matmul(ps_psq[h], ones_mat,
                                         sq7[:, h * 512:(h + 1) * 512],
                                         start=False, stop=True)
                    for h in range(2):
                        nc.vector.tensor_scalar(psqn[:, h * 512:(h + 1) * 512],
                                                ps_psq[h][0:1, :], -0.25, None,
                                                op0=AOT.mult)
                    psup.release()
                    psp2[0] = tc.alloc_tile_pool(name=f"psp2{rep}", bufs=1,
                                                 space="PSUM")
                    open_tile(3)

                    for i in range(NT):
                        pd = pds.pop(i)
                        for h in range(2):
                            nc.tensor.matmul(
                                pd[:, h * 512:(h + 1) * 512],
                                embT8[:, 6:8, i * 128:(i + 1) * 128],
                                np8[:, 6:8, h * 512:(h + 1) * 512],
                                start=False, stop=False, perf_mode=DR)
                            nc.tensor.matmul(
                                pd[:, h * 512:(h + 1) * 512], ones_row,
                                psqn[:, h * 512:(h + 1) * 512],
                                start=False, stop=True)
                        ot = outp.tile([128, C], F16, tag="ot")
                        if i % 2 == 0:
                            nc.vector.tensor_scalar(ot, pd, esq[:, i:i + 1],
                                                    None, op0=AOT.subtract)
                        else:
                            nc.scalar.activation(ot, pd, ACTF.Identity,
                                                 bias=esqn[:, i:i + 1])
                        e = nc.sync if i % 2 == 0 else nc.gpsimd
                        e.dma_start(out[i * 128:(i + 1) * 128, :], ot)
                        # late esq tiles + their negation, interleaved so the
                        # ScalarE queue never blocks an upcoming evac
                        if i in (1, 3, 5, 7):
                            t = 28 + (i - 1) // 2
                            scr8 = scrp.tile([128, D], F8, tag="scr8")
                            nc.scalar.activation(scr8, emb8[:, t, :],
                                                 ACTF.Square,
                                                 accum_out=esq[:, t:t + 1])
                        if i == 8:
                            nc.vector.tensor_scalar(esqn[:, 24:32],
                                                    esq[:, 24:32], -1.0,
                                                    None, op0=AOT.mult)
                        if i + 4 < NT:
                            open_tile(i + 4)
                    psp2[0].release()
                    psp.release()

    nc.compile()
    return nc


_NC_CACHE = None


def _get_nc():
    global _NC_CACHE
    if _NC_CACHE is None:
        _NC_CACHE = build()
    return _NC_CACHE


def make_in_maps(embeddings, prototypes, counter, y_true):
    f8 = ml_dtypes.float8_e4m3fn
    embf = np.asarray(embeddings, dtype=np.float32)
    emb8 = embf.astype(f8)
    proto16 = np.asarray(prototypes, dtype=np.float32).astype(np.float16)
    ptT = np.ascontiguousarray(
        proto16.T.reshape(KB, 128, C).transpose(1, 0, 2).reshape(128, KB * C))
    counter = np.ascontiguousarray(
        np.asarray(counter, dtype=np.float32).astype(np.float16))
    y_true = np.ascontiguousarray(np.asarray(y_true).astype(np.int32))
    in_maps = []
    for i in range(N_CORES):
        sl = slice(i * N_SHARD, (i + 1) * N_SHARD)
        e8c = emb8[sl]
        eT = np.ascontiguousarray(
            e8c.T.reshape(KB, 128, N_SHARD).transpose(1, 0, 2).reshape(
                128, KB * N_SHARD))
        in_maps.append({
            "emb8": np.ascontiguousarray(e8c),
            "embT8": eT,
            "y": y_true[sl],
            "ptT": ptT,
            "counter": counter,
        })
    return in_maps


def kernel(embeddings, prototypes, counter, y_true):
    nc = _get_nc()
    in_maps = make_in_maps(embeddings, prototypes, counter, y_true)
    res = bass_utils.run_bass_kernel_spmd(nc, in_maps,
                                          core_ids=list(range(N_CORES)))
    return np.concatenate(
        [res.results[i]["out"] for i in range(N_CORES)], axis=0
    ).astype(np.float32)
